# revision 10
# baseline (speedup 1.0000x reference)
"""Single-head causal attention on 8 TRN2 NeuronCores (one batch element per core).

Reference computation (per batch b):
  q = x@Wq, k = x@Wk, v = x@Wv          [T,H], T=2048, C=1024, H=64
  S = q k^T / sqrt(C), causal mask, softmax rows, out = P v

Device dataflow (per core, x := x[b] [T, C]):
  1. PE-transpose x 128x128 blocks -> xT [C, T] in SBUF (fp32r, exact).
  2. Projections: qk^T psum [128, 512] = [Wq|Wk]_kc^T-stacked lhsT @ xT
     chunks (contract C); v^T likewise. All fp32r, N=512 (full PE rate).
  3. Per 512-wide t-chunk c: S^T s-tiles [128,512] = k^T-slice lhsT @ q^T
     (contract H=64); exp on ACT with scale=1/32 folded in; causal mask via
     multiply with host 0/1 masks on the 4 diagonal tiles; accumulate
     O^T [65,512] += V''_k lhsT @ P^T_k where V'' = [v; ones] (row 64 of the
     rhs-transposed v gives softmax denominators for free).
  4. PE-transpose O^T back to [128, 65] tiles, divide by row sums
     (DVE reciprocal + ACT copy*scale), DMA out.
Only lower-triangle s-tiles are ever computed.
"""
import numpy as np

B, T, C, H = 8, 2048, 1024, 64
KC = C // 128          # 8 contraction chunks
NCH = T // 512         # 4 t-chunks
SCALE = 1.0 / np.sqrt(C)


def _build_program(xt_eng="dve", pss_bufs=4, pt_bufs=6, psot_bufs=2, phases=3, xin_bufs=4, pst_bufs=4, psqk_bufs=2, psv_bufs=2, xsplit=2):
    import concourse.bacc as bacc
    import concourse.tile as tile
    from concourse import mybir

    f32 = mybir.dt.float32
    f32r = mybir.dt.float32r
    Exp = mybir.ActivationFunctionType.Exp
    Copy = mybir.ActivationFunctionType.Copy

    nc = bacc.Bacc("TRN2", target_bir_lowering=False, debug=False, num_devices=B)
    x_d = nc.dram_tensor("x", [T, C], f32r, kind="ExternalInput").ap()
    wqk_d = nc.dram_tensor("wqk", [KC, 128, 128], f32r, kind="ExternalInput").ap()
    wv_d = nc.dram_tensor("wv", [KC, 128, H], f32r, kind="ExternalInput").ap()
    masks_d = nc.dram_tensor("masks", [4, 128, 512], f32r, kind="ExternalInput").ap()
    ones_d = nc.dram_tensor("ones", [1, T], f32r, kind="ExternalInput").ap()
    idn_d = nc.dram_tensor("idn", [128, 128], f32, kind="ExternalInput").ap()
    idnr_d = nc.dram_tensor("idnr", [128, 128], f32r, kind="ExternalInput").ap()
    out_d = nc.dram_tensor("out", [T, H], f32, kind="ExternalOutput").ap()

    TT = T // 128  # 16 row tiles

    with tile.TileContext(nc) as tc:
        with (
            tc.tile_pool(name="const", bufs=1) as cpool,
            tc.tile_pool(name="big", bufs=1) as big,
            tc.tile_pool(name="pt", bufs=pt_bufs) as ptp,
            tc.tile_pool(name="outp", bufs=3) as outp,
        ):
            idn = cpool.tile([128, 128], f32, tag="idn")
            nc.sync.dma_start(idn[:], idn_d)
            idnr = cpool.tile([128, 128], f32r, tag="idnr")
            nc.sync.dma_start(idnr[:], idnr_d)
            wqk = cpool.tile([128, KC * 128], f32r, tag="wqk")
            wv = cpool.tile([128, KC * H], f32r, tag="wv")
            for kc in range(KC):
                nc.sync.dma_start(wqk[:, kc * 128:(kc + 1) * 128], wqk_d[kc])
                nc.sync.dma_start(wv[:, kc * H:(kc + 1) * H], wv_d[kc])
            masks = cpool.tile([128, 4 * 512], f32r, tag="masks")
            for j in range(4):
                nc.sync.dma_start(masks[:, j * 512:(j + 1) * 512], masks_d[j])

            # xT[c, t] laid out as 8 chunks side by side: col kc*T + t
            xT = big.tile([128, KC * T], f32r, tag="xT")
            qT = big.tile([64, T], f32r, tag="qT")
            kT = big.tile([64, T], f32r, tag="kT")
            vTa = big.tile([128, T], f32r, tag="vTa")  # v^T, ones at row 64, rest unused
            nc.sync.dma_start(vTa[64:65, :], ones_d)
            vpp = big.tile([128, TT * 72], f32r, tag="vpp")  # 16x [128,65] slots

            # ---- Phase 1: load x tiles, transpose to xT ----
            with (
                tc.tile_pool(name="xin", bufs=xin_bufs) as xinp,
                tc.tile_pool(name="pst", bufs=pst_bufs, space="PSUM") as pstp,
                tc.tile_pool(name="psqk", bufs=psqk_bufs, space="PSUM") as psqkp,
                tc.tile_pool(name="psv", bufs=psv_bufs, space="PSUM") as psvp,
            ):
                xTv = xT[:].rearrange("p (kc t) -> p kc t", kc=KC)
                for tt in range(TT):
                    xin = xinp.tile([128, C], f32r, tag="xin")
                    for sp in range(xsplit):
                        w = C // xsplit
                        eng = nc.sync if (tt * xsplit + sp) % 2 == 0 else nc.scalar
                        eng.dma_start(
                            xin[:, sp * w:(sp + 1) * w],
                            x_d[tt * 128:(tt + 1) * 128, sp * w:(sp + 1) * w])
                    for g in range(KC // 4):
                        tp = pstp.tile([128, 512], f32r, tag="tp")
                        for u in range(4):
                            kc = g * 4 + u
                            nc.tensor.transpose(
                                tp[:, u * 128:(u + 1) * 128],
                                xin[:, kc * 128:(kc + 1) * 128], idnr[:]
                            )
                        dst = xTv[:, g * 4:(g + 1) * 4, tt * 128:(tt + 1) * 128]
                        src = tp[:].rearrange("p (u t) -> p u t", u=4)
                        if (tt * 2 + g) % 2 == 0:
                            nc.vector.tensor_copy(dst, src)
                        else:
                            nc.scalar.activation(dst, src, Copy)

                # ---- Phase 2: projections per t-chunk ----
                for c in range(NCH if phases >= 2 else 0):
                    qkps = psqkp.tile([128, 512], f32, tag="qkps")
                    vps = psvp.tile([64, 512], f32, tag="vps")
                    for kc in range(KC):
                        rhs = xT[:, kc * T + c * 512: kc * T + c * 512 + 512]
                        nc.tensor.matmul(
                            qkps[:], wqk[:, kc * 128:(kc + 1) * 128], rhs,
                            start=(kc == 0), stop=(kc == KC - 1),
                        )
                        nc.tensor.matmul(
                            vps[:], wv[:, kc * H:(kc + 1) * H], rhs,
                            start=(kc == 0), stop=(kc == KC - 1),
                        )
                    sl = slice(c * 512, (c + 1) * 512)
                    nc.vector.tensor_copy(qT[:, sl], qkps[0:64, :])
                    nc.vector.tensor_copy(kT[:, sl], qkps[64:128, :])
                    nc.vector.tensor_copy(vTa[0:64, sl], vps[:])

                # ---- Phase 2b: V'' tiles = transpose of vTa blocks ----
                for tt in range(TT if phases >= 2 else 0):
                    vtp = pstp.tile([128, 128], f32r, tag="tp")
                    nc.tensor.transpose(
                        vtp[:], vTa[:, tt * 128:(tt + 1) * 128], idnr[:]
                    )
                    nc.vector.tensor_copy(
                        vpp[:, tt * 72: tt * 72 + 65], vtp[:, 0:65]
                    )

            # ---- Phase 3: attention per t-chunk ----
            if phases < 3:
                import contextlib
                raise_ctx = None
            with (
                tc.tile_pool(name="pss", bufs=pss_bufs, space="PSUM") as pssp,
                tc.tile_pool(name="psO", bufs=2, space="PSUM") as psOp,
                tc.tile_pool(name="psot", bufs=psot_bufs, space="PSUM") as psotp,
            ):
                for c in range(NCH if phases >= 3 else 0):
                    oTps = psOp.tile([65, 512], f32, tag="oTps")
                    nkt = 4 * c + 4
                    for k in range(nkt):
                        sps = pssp.tile([128, 512], f32, tag="sps")
                        nc.tensor.matmul(
                            sps[:], kT[:, k * 128:(k + 1) * 128],
                            qT[:, c * 512:(c + 1) * 512],
                            start=True, stop=True,
                        )
                        pT = ptp.tile([128, 512], f32r, tag="pT")
                        nc.scalar.activation(pT[:], sps[:], Exp, scale=SCALE)
                        if k >= 4 * c:
                            j = k - 4 * c
                            nc.vector.tensor_mul(
                                pT[:], pT[:], masks[:, j * 512:(j + 1) * 512]
                            )
                        nc.tensor.matmul(
                            oTps[:], vpp[:, k * 72: k * 72 + 65], pT[:],
                            start=(k == 0), stop=(k == nkt - 1),
                        )
                    oT = outp.tile([128, 512], f32, tag="oT")
                    nc.scalar.activation(oT[0:65, :], oTps[:], Copy)
                    for j in range(4):
                        otps = psotp.tile([128, 128], f32, tag="otps")
                        nc.tensor.transpose(
                            otps[:], oT[:, j * 128:(j + 1) * 128], idn[:]
                        )
                        rec = outp.tile([128, 1], f32, tag="rec")
                        nc.vector.reciprocal(rec[:], otps[:, 64:65])
                        osb = outp.tile([128, H], f32, tag="osb")
                        nc.scalar.activation(
                            osb[:], otps[:, 0:H], Copy, scale=rec[:]
                        )
                        tt = c * 4 + j
                        nc.sync.dma_start(
                            out_d[tt * 128:(tt + 1) * 128, :], osb[:]
                        )
    nc.compile()
    return nc


_CACHED = {}


def _prep_shared(Wq, Wk, Wv):
    wqk = np.stack([
        np.concatenate([Wq[kc * 128:(kc + 1) * 128], Wk[kc * 128:(kc + 1) * 128]],
                       axis=1)
        for kc in range(KC)
    ]).astype(np.float32)
    wv = np.stack([Wv[kc * 128:(kc + 1) * 128] for kc in range(KC)]).astype(np.float32)
    ds, dt = np.arange(128)[:, None], np.arange(512)[None, :]
    masks = np.stack([(ds + 128 * j <= dt).astype(np.float32) for j in range(4)])
    ones = np.ones((1, T), dtype=np.float32)
    idn = np.eye(128, dtype=np.float32)
    return wqk, wv, masks, ones, idn


def _run(x, Wq, Wk, Wv, trace=False):
    from concourse.bass_utils import run_bass_kernel_spmd

    if "nc" not in _CACHED:
        _CACHED["nc"] = _build_program()
    nc = _CACHED["nc"]
    wqk, wv, masks, ones, idn = _prep_shared(
        np.asarray(Wq, np.float32), np.asarray(Wk, np.float32),
        np.asarray(Wv, np.float32))
    x = np.asarray(x, np.float32)
    in_maps = [
        {"x": np.ascontiguousarray(x[b]), "wqk": wqk, "wv": wv, "masks": masks,
         "ones": ones, "idn": idn, "idnr": idn}
        for b in range(B)
    ]
    res = run_bass_kernel_spmd(nc, in_maps, core_ids=list(range(B)), trace=trace)
    out = np.stack([res.results[b]["out"] for b in range(B)])
    return out, res


def kernel(x, Wq, Wk, Wv):
    out, _ = _run(x, Wq, Wk, Wv)
    return out
